# revision 26
# baseline (speedup 1.0000x reference)
"""Trainium2 Bass kernel for the CMB power-spectrum emulator problem.

Math: a 4-layer MLP maps phi (512,2) -> diag (128 knots, 512 ch); a natural
cubic spline through the 128 knots is evaluated on a constant 256x256
isotropic-frequency grid, then exp(.)*NORM.

Structural collapses (all input-independent):
 1. The spline is linear in the knot values, so the whole spline stage is
    one constant matrix E:  out = exp(E @ diag + ln NORM).
 2. wn_iso[i,j] depends only on s = a^2 + b^2 with (a,b) = (|wn_i|,|wn_j|):
    only 5924 of the 65536 grid points are distinct VALUES, and equal values
    produce bitwise-equal outputs. The device computes the 5952 (padded)
    unique points; the host replicates them with a constant gather.
 3. The natural cubic spline reproduces constants exactly (E rows sum to 1),
    so ln NORM is folded into b4 on the host: out = exp(E @ diag'), which
    frees the exp activation from any SBUF bias operand.

Device work per core (unique-value sharding, 744 points/core, 512 ch):
  MLP as two interleaved 256-wide chains (f32r matmuls on TensorE,
    relu+bias and the final bias-add on the otherwise-idle VectorE)
  per 128-channel group g: psum = diag_g.T @ ET_u  (TensorE, f32r)
                           stage = exp(psum)       (ScalarE LUT, ~2 ULP)
                           store (128, 744) bf16   (SP HWDGE ring)
  Loads are split over both HWDGE rings (params on SP, ET on ACT) so the
  MLP is never gated on the big ET transfer.
"""

import os

import numpy as np

B = 512
N_CORES = 8
N_UNIQ = 5924                 # distinct wn_iso values on the grid
P_CORE = 744                  # per-core unique points (8 x 744 = 5952 padded)
P_PAD = N_CORES * P_CORE
NORM = 1.0 / 12661.0

MIN_PHI = np.array([50.0, 0.0075], np.float32)
DPHI = np.array([40.0, 0.0492], np.float32)
MU = np.array([70.0, 0.032], np.float32)
SIG = np.array([20.0, 0.025], np.float32)

# matmul dtype: "f32" (4 cyc/row, exact), "f32r" (1 cyc/row, ~19-bit mantissa)
MODE = os.environ.get("BASS_KERNEL_MODE", "f32r")

# packed parameters: pp (2 partitions: phiT|W1), pk (128p: W2|W3|W4),
# pb (128p fp32: b1|b2|b3|b4', with ln(NORM) folded into b4')
PP_COLS = 612
PK_COLS = 328
PB_COLS = 4

_CACHE = {}


def _spline_eval_matrix(wn_vals):
    """E (len(wn_vals), 128) fp32: natural-cubic-spline evaluation at wn_vals,
    linear in the 128 knot values (knots t_k = sqrt(2)*k in fp32)."""
    wn = (256.0 * np.fft.fftfreq(256, d=1.0)).reshape(256, 1)
    wn_iso = np.sqrt(wn**2 + wn.reshape(1, 256) ** 2)
    t32 = np.fft.fftshift(wn_iso).diagonal()[128:].astype(np.float32)  # (128,)

    n = 128
    t = t32.astype(np.float64)
    h = np.diff(t)
    A = np.diag(2.0 * (h[:-1] + h[1:])) + np.diag(h[1:-1], 1) + np.diag(h[1:-1], -1)
    D1 = np.zeros((n - 1, n))
    for i in range(n - 1):
        D1[i, i] = -1.0 / h[i]
        D1[i, i + 1] = 1.0 / h[i]
    D2 = 6.0 * (D1[1:] - D1[:-1])
    L = np.zeros((n, n))
    L[1:-1] = np.linalg.solve(A, D2)

    Sa = np.eye(n)[: n - 1]
    Sb = D1 - (h[:, None] / 6.0) * (2.0 * L[:-1] + L[1:])
    Sc = L[:-1] / 2.0
    Sd = (L[1:] - L[:-1]) / (6.0 * h[:, None])

    w32 = wn_vals.astype(np.float32)
    idx = np.clip(np.searchsorted(t32, w32, side="right") - 1, 0, n - 2)
    f = (w32 - t32[idx]).astype(np.float64)[:, None]
    E = Sa[idx] + f * (Sb[idx] + f * (Sc[idx] + f * Sd[idx]))
    return E.astype(np.float32)


def _build_constants():
    """ET_u (128, P_PAD) fp32 for the unique values, and IDX (65536,) int32
    mapping each full-grid point to its unique-value column."""
    k = np.arange(256)
    absw = np.minimum(k, 256 - k)  # |wn_i|, with |wn_0| = 0, |wn_128| = 128
    s = absw[:, None].astype(np.int64) ** 2 + absw[None, :].astype(np.int64) ** 2
    uniq_s, inv = np.unique(s.ravel(), return_inverse=True)  # (N_UNIQ,), (65536,)

    wn_vals = np.sqrt(uniq_s.astype(np.float64))
    E = _spline_eval_matrix(wn_vals)  # (N_UNIQ, 128)
    ET = np.zeros((128, P_PAD), np.float32)
    ET[:, :N_UNIQ] = E.T
    return np.ascontiguousarray(ET), inv.astype(np.int32)


def _build_program(mode):
    import concourse.bass as bass
    import concourse.bacc as bacc
    import concourse.mybir as mybir
    from concourse import tile

    f32 = mybir.dt.float32
    bf16 = mybir.dt.bfloat16
    mm_dt = {"f32r": mybir.dt.float32r, "f32": f32}[mode]
    nc = bacc.Bacc("TRN2", target_bir_lowering=False, debug=False)

    pp_d = nc.dram_tensor("pp", [2, PP_COLS], mm_dt, kind="ExternalInput")
    pk_d = nc.dram_tensor("pk", [128, PK_COLS], mm_dt, kind="ExternalInput")
    pb_d = nc.dram_tensor("pb", [128, PB_COLS], f32, kind="ExternalInput")
    et_d = nc.dram_tensor("et", [128, P_CORE], mm_dt, kind="ExternalInput")
    out_d = nc.dram_tensor("out", [128, 4 * P_CORE], bf16, kind="ExternalOutput")

    Relu = mybir.ActivationFunctionType.Relu
    Exp = mybir.ActivationFunctionType.Exp

    N_GRP = 4
    SUB = 512  # matmul free chunk (f32 moving-operand limit)
    HBW = 256  # MLP half-batch width (psum tile size)

    with tile.TileContext(nc) as tc:
        with (
            tc.tile_pool(name="const", bufs=1) as cpool,
            tc.tile_pool(name="mlp", bufs=2) as mpool,
            tc.tile_pool(name="stage", bufs=4) as spool,
            tc.tile_pool(name="psum", bufs=2, space=bass.MemorySpace.PSUM) as ppool,
            tc.tile_pool(name="mpsum", bufs=2, space=bass.MemorySpace.PSUM) as mps,
        ):
            # ---- loads: pk+stores on the SP ring; phi, biases and ET on the
            # ACT ring (phi first so layer 1 starts earliest; pk lands in
            # parallel on the other ring well before layer 2 needs it) ----
            pk_t = cpool.tile([128, PK_COLS], mm_dt, tag="pk")
            nc.sync.dma_start(pk_t[:], pk_d[:])

            # warm-up matmuls on zeros: keeps the PE busy through the load
            # window so the HAM clock gate releases (1.2 -> 2.4 GHz) before
            # the real matmuls run
            z_t = cpool.tile([128, 512], f32, tag="z")
            nc.vector.memset(z_t[:], 0.0)
            z_r = cpool.tile([128, 256], mm_dt, tag="zr")
            nc.vector.tensor_scalar(
                z_r[:], z_t[:, 0:256], 0.0, None, mybir.AluOpType.add
            )
            zp = mps.tile([128, HBW], f32, tag="mps")
            for _ in range(3):
                nc.tensor.matmul(zp[:], z_t[0:128, 0:128], z_t[:, 0:HBW])
            pp_t = cpool.tile([2, PP_COLS], mm_dt, tag="pp")
            nc.scalar.dma_start(pp_t[:], pp_d[:])
            pb_t = cpool.tile([128, PB_COLS], f32, tag="pb")
            nc.scalar.dma_start(pb_t[:], pb_d[:])
            et_t = cpool.tile([128, P_CORE], mm_dt, tag="et")
            nc.scalar.dma_start(et_t[:], et_d[:])

            pht = pp_t[0:2, 0:512]
            w1 = pp_t[0:2, 512:612]
            w2 = pk_t[0:100, 0:100]
            w3 = pk_t[0:100, 100:200]
            w4 = pk_t[0:100, 200:328]
            b1 = pb_t[0:100, 0:1]
            b2 = pb_t[0:100, 1:2]
            b3 = pb_t[0:100, 2:3]
            b4 = pb_t[0:128, 3:4]  # includes ln(NORM) fold

            # ---- MLP, two interleaved 256-wide chains (hides sem latency) ----
            HB = B // 2
            diag = mpool.tile([128, B], mm_dt, tag="diag")
            hs = {}
            for lyr, (wt, bt, act, win, wout) in enumerate(
                [
                    (w1, b1, Relu, 2, 100),
                    (w2, b2, Relu, 100, 100),
                    (w3, b3, Relu, 100, 100),
                    (w4, b4, None, 100, 128),
                ]
            ):
                for c in range(2):
                    cs = slice(c * HB, (c + 1) * HB)
                    src = pht[:, cs] if lyr == 0 else hs[c][:]
                    ps = mps.tile([128, HBW], f32, tag="mps")
                    nc.tensor.matmul(ps[0:wout, 0:HB], wt, src)
                    if lyr < 3:
                        h = mpool.tile([100, HB], mm_dt, tag=f"h{lyr}{c}")
                        nc.vector.tensor_scalar(
                            h[:], ps[0:wout, 0:HB], bt, 0.0,
                            mybir.AluOpType.add, mybir.AluOpType.max,
                        )
                        hs[c] = h
                    else:
                        nc.vector.tensor_scalar(
                            diag[:, cs], ps[0:wout, 0:HB], bt, None,
                            mybir.AluOpType.add,
                        )
                # f32r filler in the relu-wait gap: keeps PE activity
                # sustained so the HAM gate climbs to the top clock level
                zf = mps.tile([128, HBW], f32, tag="zf")
                nc.tensor.matmul(zf[:], z_r[0:128, 0:128], z_r[:])

            # ---- main: out[g] = exp(diag_g.T @ ET_u), one bf16 store per g.
            # Filler matmuls plug the PE idle gaps (psum WAR on exp) so the
            # HAM clock gate stays released through the group pipeline. ----
            for g in range(N_GRP):
                ps = ppool.tile([128, P_CORE], f32, tag="ps")
                for off in range(0, P_CORE, SUB):
                    w = min(SUB, P_CORE - off)
                    nc.tensor.matmul(
                        ps[:, off : off + w],
                        diag[:, g * 128 : (g + 1) * 128],
                        et_t[:, off : off + w],
                    )
                stage = spool.tile([128, P_CORE], bf16, tag="stage")
                nc.scalar.activation(stage[:], ps[:], Exp)
                nc.sync.dma_start(out_d[:, g * P_CORE : (g + 1) * P_CORE], stage[:])
                if g in (1, 2):
                    zf = mps.tile([128, HBW], f32, tag="zf")
                    nc.tensor.matmul(zf[:], z_r[0:128, 0:128], z_r[:])

    nc.compile()
    return nc


def _get_cached():
    key = ("nc", MODE)
    if key not in _CACHE:
        _CACHE[key] = _build_program(MODE)
    if "consts" not in _CACHE:
        _CACHE["consts"] = _build_constants()
    return (_CACHE[key],) + _CACHE["consts"]


def _make_in_maps(phi, W1, b1, W2, b2, W3, b3, W4, b4, ET):
    # fold the input normalization into the first layer, ln(NORM) into b4
    scale = (DPHI / SIG).astype(np.float32)
    shift = ((MIN_PHI - MU) / SIG).astype(np.float32)
    W1f = (np.asarray(W1, np.float32) * scale[:, None]).astype(np.float32)
    b1f = (np.asarray(b1, np.float32) + shift @ np.asarray(W1, np.float32)).astype(
        np.float32
    )

    pp = np.zeros((2, PP_COLS), np.float32)
    pp[:, 0:512] = np.asarray(phi, np.float32).T
    pp[:, 512:612] = W1f
    pk = np.zeros((128, PK_COLS), np.float32)
    pk[0:100, 0:100] = np.asarray(W2, np.float32)
    pk[0:100, 100:200] = np.asarray(W3, np.float32)
    pk[0:100, 200:328] = np.asarray(W4, np.float32)
    pb = np.zeros((128, PB_COLS), np.float32)
    pb[0:100, 0] = b1f
    pb[0:100, 1] = np.asarray(b2, np.float32)
    pb[0:100, 2] = np.asarray(b3, np.float32)
    pb[0:128, 3] = np.asarray(b4, np.float32) + np.float32(
        np.log(np.float64(NORM))
    )

    common = {"pp": pp, "pk": pk, "pb": pb}
    in_maps = []
    for c in range(N_CORES):
        m = dict(common)
        m["et"] = np.ascontiguousarray(ET[:, c * P_CORE : (c + 1) * P_CORE])
        in_maps.append(m)
    return in_maps


def kernel(phi, W1, b1, W2, b2, W3, b3, W4, b4):
    from concourse.bass_utils import run_bass_kernel_spmd

    nc, ET, IDX = _get_cached()
    in_maps = _make_in_maps(phi, W1, b1, W2, b2, W3, b3, W4, b4, ET)
    res = run_bass_kernel_spmd(nc, in_maps, core_ids=list(range(N_CORES)))
    uniq = np.empty((B, P_PAD), np.float32)
    for c, r in enumerate(res.results):
        o = np.asarray(r["out"]).astype(np.float32)  # (128, 4*P_CORE) bf16
        for g in range(4):
            uniq[g * 128 : (g + 1) * 128, c * P_CORE : (c + 1) * P_CORE] = o[
                :, g * P_CORE : (g + 1) * P_CORE
            ]
    full = np.take(uniq, IDX, axis=1)  # (512, 65536) constant-gather replication
    return np.ascontiguousarray(full.reshape(B, 256, 256))


# revision 30
# speedup vs baseline: 1.0510x; 1.0510x over previous
"""Trainium2 Bass kernel for the CMB power-spectrum emulator problem.

Math: a 4-layer MLP maps phi (512,2) -> diag (128 knots, 512 ch); a natural
cubic spline through the 128 knots is evaluated on a constant 256x256
isotropic-frequency grid, then exp(.)*NORM.

Structural collapses (all input-independent):
 1. The spline is linear in the knot values, so the whole spline stage is
    one constant matrix E:  out = exp(E @ diag + ln NORM).
 2. wn_iso[i,j] depends only on s = a^2 + b^2 with (a,b) = (|wn_i|,|wn_j|):
    only 5924 of the 65536 grid points are distinct VALUES, and equal values
    produce bitwise-equal outputs. The device computes the 5952 (padded)
    unique points; the host replicates them with a constant gather.
 3. The natural cubic spline reproduces constants exactly (E rows sum to 1),
    so ln NORM is folded into b4 on the host: out = exp(E @ diag'), which
    frees the exp activation from any SBUF bias operand.

Device work per core (unique-value sharding, 744 points/core, 512 ch):
  MLP as two interleaved 256-wide chains (f32r matmuls on TensorE,
    relu+bias and the final bias-add on the otherwise-idle VectorE)
  per 128-channel group g: psum = diag_g.T @ ET_u  (TensorE, f32r)
                           stage = exp(psum)       (ScalarE LUT, ~2 ULP)
                           store (128, 744) bf16   (SP HWDGE ring)
  Loads are split over both HWDGE rings (params on SP, ET on ACT) so the
  MLP is never gated on the big ET transfer.
"""

import os

import numpy as np

B = 512
N_CORES = 8
N_UNIQ = 5924                 # distinct wn_iso values on the grid
P_CORE = 744                  # per-core unique points (8 x 744 = 5952 padded)
P_PAD = N_CORES * P_CORE
NORM = 1.0 / 12661.0

MIN_PHI = np.array([50.0, 0.0075], np.float32)
DPHI = np.array([40.0, 0.0492], np.float32)
MU = np.array([70.0, 0.032], np.float32)
SIG = np.array([20.0, 0.025], np.float32)

# matmul dtype: "f32" (4 cyc/row, exact), "f32r" (1 cyc/row, ~19-bit mantissa)
MODE = os.environ.get("BASS_KERNEL_MODE", "f32r")

# packed parameters: pp (2 partitions: phiT|W1), pk (128p: W2|W3|W4),
# pb (128p fp32: b1|b2|b3|b4', with ln(NORM) folded into b4')
PP_COLS = 612
PK_COLS = 328
PB_COLS = 4

_CACHE = {}


def _spline_eval_matrix(wn_vals):
    """E (len(wn_vals), 128) fp32: natural-cubic-spline evaluation at wn_vals,
    linear in the 128 knot values (knots t_k = sqrt(2)*k in fp32)."""
    wn = (256.0 * np.fft.fftfreq(256, d=1.0)).reshape(256, 1)
    wn_iso = np.sqrt(wn**2 + wn.reshape(1, 256) ** 2)
    t32 = np.fft.fftshift(wn_iso).diagonal()[128:].astype(np.float32)  # (128,)

    n = 128
    t = t32.astype(np.float64)
    h = np.diff(t)
    A = np.diag(2.0 * (h[:-1] + h[1:])) + np.diag(h[1:-1], 1) + np.diag(h[1:-1], -1)
    D1 = np.zeros((n - 1, n))
    for i in range(n - 1):
        D1[i, i] = -1.0 / h[i]
        D1[i, i + 1] = 1.0 / h[i]
    D2 = 6.0 * (D1[1:] - D1[:-1])
    L = np.zeros((n, n))
    L[1:-1] = np.linalg.solve(A, D2)

    Sa = np.eye(n)[: n - 1]
    Sb = D1 - (h[:, None] / 6.0) * (2.0 * L[:-1] + L[1:])
    Sc = L[:-1] / 2.0
    Sd = (L[1:] - L[:-1]) / (6.0 * h[:, None])

    w32 = wn_vals.astype(np.float32)
    idx = np.clip(np.searchsorted(t32, w32, side="right") - 1, 0, n - 2)
    f = (w32 - t32[idx]).astype(np.float64)[:, None]
    E = Sa[idx] + f * (Sb[idx] + f * (Sc[idx] + f * Sd[idx]))
    return E.astype(np.float32)


def _build_constants():
    """ET_u (128, P_PAD) fp32 for the unique values, and IDX (65536,) int32
    mapping each full-grid point to its unique-value column."""
    k = np.arange(256)
    absw = np.minimum(k, 256 - k)  # |wn_i|, with |wn_0| = 0, |wn_128| = 128
    s = absw[:, None].astype(np.int64) ** 2 + absw[None, :].astype(np.int64) ** 2
    uniq_s, inv = np.unique(s.ravel(), return_inverse=True)  # (N_UNIQ,), (65536,)

    wn_vals = np.sqrt(uniq_s.astype(np.float64))
    E = _spline_eval_matrix(wn_vals)  # (N_UNIQ, 128)
    ET = np.zeros((128, P_PAD), np.float32)
    ET[:, :N_UNIQ] = E.T
    return np.ascontiguousarray(ET), inv.astype(np.int32)


def _build_program(mode):
    import concourse.bass as bass
    import concourse.bacc as bacc
    import concourse.mybir as mybir
    from concourse import tile

    f32 = mybir.dt.float32
    bf16 = mybir.dt.bfloat16
    mm_dt = {"f32r": mybir.dt.float32r, "f32": f32}[mode]
    nc = bacc.Bacc("TRN2", target_bir_lowering=False, debug=False)

    pp_d = nc.dram_tensor("pp", [2, PP_COLS], mm_dt, kind="ExternalInput")
    pk_d = nc.dram_tensor("pk", [128, PK_COLS], mm_dt, kind="ExternalInput")
    pb_d = nc.dram_tensor("pb", [128, PB_COLS], f32, kind="ExternalInput")
    et_d = nc.dram_tensor("et", [128, P_CORE], mm_dt, kind="ExternalInput")
    out_d = nc.dram_tensor("out", [128, 4 * P_CORE], bf16, kind="ExternalOutput")

    Relu = mybir.ActivationFunctionType.Relu
    Exp = mybir.ActivationFunctionType.Exp

    N_GRP = 4
    SUB = 512  # matmul free chunk (f32 moving-operand limit)
    HBW = 256  # MLP half-batch width (psum tile size)

    with tile.TileContext(nc) as tc:
        with (
            tc.tile_pool(name="const", bufs=1) as cpool,
            tc.tile_pool(name="mlp", bufs=2) as mpool,
            tc.tile_pool(name="stage", bufs=4) as spool,
            tc.tile_pool(name="psum", bufs=2, space=bass.MemorySpace.PSUM) as ppool,
            tc.tile_pool(name="mpsum", bufs=2, space=bass.MemorySpace.PSUM) as mps,
        ):
            # ---- loads: pk+stores on the SP ring; phi, biases and ET on the
            # ACT ring (phi first so layer 1 starts earliest; pk lands in
            # parallel on the other ring well before layer 2 needs it) ----
            pk_t = cpool.tile([128, PK_COLS], mm_dt, tag="pk")
            nc.sync.dma_start(pk_t[:], pk_d[:])

            # warm-up matmuls on zeros: keeps the PE busy through the load
            # window so the HAM clock gate releases (1.2 -> 2.4 GHz) before
            # the real matmuls run
            z_t = cpool.tile([128, 256], f32, tag="z")
            nc.gpsimd.memset(z_t[:], 0.0)
            z_r = cpool.tile([128, 256], mm_dt, tag="zr")
            nc.vector.tensor_scalar(
                z_r[:], z_t[:], 0.0, None, mybir.AluOpType.add
            )
            zp = mps.tile([128, HBW], f32, tag="mps")
            for _ in range(3):
                nc.tensor.matmul(zp[:], z_t[0:128, 0:128], z_t[:])
            pp_t = cpool.tile([2, PP_COLS], mm_dt, tag="pp")
            nc.scalar.dma_start(pp_t[:], pp_d[:])
            pb_t = cpool.tile([128, PB_COLS], f32, tag="pb")
            nc.scalar.dma_start(pb_t[:], pb_d[:])
            et_t = cpool.tile([128, P_CORE], mm_dt, tag="et")
            nc.scalar.dma_start(et_t[:], et_d[:])

            pht = pp_t[0:2, 0:512]
            w1 = pp_t[0:2, 512:612]
            w2 = pk_t[0:100, 0:100]
            w3 = pk_t[0:100, 100:200]
            w4 = pk_t[0:100, 200:328]
            b1 = pb_t[0:100, 0:1]
            b2 = pb_t[0:100, 1:2]
            b3 = pb_t[0:100, 2:3]
            b4 = pb_t[0:128, 3:4]  # includes ln(NORM) fold

            # ---- MLP, two interleaved 256-wide chains (hides sem latency) ----
            HB = B // 2
            diag = mpool.tile([128, B], mm_dt, tag="diag")
            hs = {}
            for lyr, (wt, bt, act, win, wout) in enumerate(
                [
                    (w1, b1, Relu, 2, 100),
                    (w2, b2, Relu, 100, 100),
                    (w3, b3, Relu, 100, 100),
                    (w4, b4, None, 100, 128),
                ]
            ):
                for c in range(2):
                    cs = slice(c * HB, (c + 1) * HB)
                    src = pht[:, cs] if lyr == 0 else hs[c][:]
                    ps = mps.tile([128, HBW], f32, tag="mps")
                    nc.tensor.matmul(ps[0:wout, 0:HB], wt, src)
                    if lyr < 3:
                        h = mpool.tile([100, HB], mm_dt, tag=f"h{lyr}{c}")
                        nc.vector.tensor_scalar(
                            h[:], ps[0:wout, 0:HB], bt, 0.0,
                            mybir.AluOpType.add, mybir.AluOpType.max,
                        )
                        hs[c] = h
                    else:
                        nc.vector.tensor_scalar(
                            diag[:, cs], ps[0:wout, 0:HB], bt, None,
                            mybir.AluOpType.add,
                        )


            # ---- main: out[g] = exp(diag_g.T @ ET_u), one bf16 store per g.
            # Filler matmuls plug the PE idle gaps (psum WAR on exp) so the
            # HAM clock gate stays released through the group pipeline. ----
            for g in range(N_GRP):
                ps = ppool.tile([128, P_CORE], f32, tag="ps")
                for off in range(0, P_CORE, SUB):
                    w = min(SUB, P_CORE - off)
                    nc.tensor.matmul(
                        ps[:, off : off + w],
                        diag[:, g * 128 : (g + 1) * 128],
                        et_t[:, off : off + w],
                    )
                stage = spool.tile([128, P_CORE], bf16, tag="stage")
                nc.scalar.activation(stage[:], ps[:], Exp)
                ring = nc.scalar if g == N_GRP - 1 else nc.sync
                ring.dma_start(out_d[:, g * P_CORE : (g + 1) * P_CORE], stage[:])
                if g in (1, 2):
                    zf = mps.tile([128, HBW], f32, tag="mps")
                    nc.tensor.matmul(
                        zf[:, 0:128], z_r[0:128, 0:128], z_r[:, 0:128]
                    )

    nc.compile()
    return nc


def _get_cached():
    key = ("nc", MODE)
    if key not in _CACHE:
        _CACHE[key] = _build_program(MODE)
    if "consts" not in _CACHE:
        _CACHE["consts"] = _build_constants()
    return (_CACHE[key],) + _CACHE["consts"]


def _make_in_maps(phi, W1, b1, W2, b2, W3, b3, W4, b4, ET):
    # fold the input normalization into the first layer, ln(NORM) into b4
    scale = (DPHI / SIG).astype(np.float32)
    shift = ((MIN_PHI - MU) / SIG).astype(np.float32)
    W1f = (np.asarray(W1, np.float32) * scale[:, None]).astype(np.float32)
    b1f = (np.asarray(b1, np.float32) + shift @ np.asarray(W1, np.float32)).astype(
        np.float32
    )

    pp = np.zeros((2, PP_COLS), np.float32)
    pp[:, 0:512] = np.asarray(phi, np.float32).T
    pp[:, 512:612] = W1f
    pk = np.zeros((128, PK_COLS), np.float32)
    pk[0:100, 0:100] = np.asarray(W2, np.float32)
    pk[0:100, 100:200] = np.asarray(W3, np.float32)
    pk[0:100, 200:328] = np.asarray(W4, np.float32)
    pb = np.zeros((128, PB_COLS), np.float32)
    pb[0:100, 0] = b1f
    pb[0:100, 1] = np.asarray(b2, np.float32)
    pb[0:100, 2] = np.asarray(b3, np.float32)
    pb[0:128, 3] = np.asarray(b4, np.float32) + np.float32(
        np.log(np.float64(NORM))
    )

    common = {"pp": pp, "pk": pk, "pb": pb}
    in_maps = []
    for c in range(N_CORES):
        m = dict(common)
        m["et"] = np.ascontiguousarray(ET[:, c * P_CORE : (c + 1) * P_CORE])
        in_maps.append(m)
    return in_maps


def kernel(phi, W1, b1, W2, b2, W3, b3, W4, b4):
    from concourse.bass_utils import run_bass_kernel_spmd

    nc, ET, IDX = _get_cached()
    in_maps = _make_in_maps(phi, W1, b1, W2, b2, W3, b3, W4, b4, ET)
    res = run_bass_kernel_spmd(nc, in_maps, core_ids=list(range(N_CORES)))
    uniq = np.empty((B, P_PAD), np.float32)
    for c, r in enumerate(res.results):
        o = np.asarray(r["out"]).astype(np.float32)  # (128, 4*P_CORE) bf16
        for g in range(4):
            uniq[g * 128 : (g + 1) * 128, c * P_CORE : (c + 1) * P_CORE] = o[
                :, g * P_CORE : (g + 1) * P_CORE
            ]
    full = np.take(uniq, IDX, axis=1)  # (512, 65536) constant-gather replication
    return np.ascontiguousarray(full.reshape(B, 256, 256))


# revision 31
# speedup vs baseline: 1.0606x; 1.0092x over previous
"""Trainium2 Bass kernel for the CMB power-spectrum emulator problem.

Math: a 4-layer MLP maps phi (512,2) -> diag (128 knots, 512 ch); a natural
cubic spline through the 128 knots is evaluated on a constant 256x256
isotropic-frequency grid, then exp(.)*NORM.

Structural collapses (all input-independent):
 1. The spline is linear in the knot values, so the whole spline stage is
    one constant matrix E:  out = exp(E @ diag + ln NORM).
 2. wn_iso[i,j] depends only on s = a^2 + b^2 with (a,b) = (|wn_i|,|wn_j|):
    only 5924 of the 65536 grid points are distinct VALUES, and equal values
    produce bitwise-equal outputs. The device computes the 5952 (padded)
    unique points; the host replicates them with a constant gather.
 3. The natural cubic spline reproduces constants exactly (E rows sum to 1),
    so ln NORM is folded into b4 on the host: out = exp(E @ diag'), which
    frees the exp activation from any SBUF bias operand.

Device work per core (unique-value sharding, 744 points/core, 512 ch):
  MLP as two interleaved 256-wide chains (f32r matmuls on TensorE,
    relu+bias and the final bias-add on the otherwise-idle VectorE)
  per 128-channel group g: psum = diag_g.T @ ET_u  (TensorE, f32r)
                           stage = exp(psum)       (ScalarE LUT, ~2 ULP)
                           store (128, 744) bf16   (SP HWDGE ring)
  Loads are split over both HWDGE rings (params on SP, ET on ACT) so the
  MLP is never gated on the big ET transfer.
"""

import os

import numpy as np

B = 512
N_CORES = 8
N_UNIQ = 5924                 # distinct wn_iso values on the grid
P_CORE = 744                  # per-core unique points (8 x 744 = 5952 padded)
P_PAD = N_CORES * P_CORE
NORM = 1.0 / 12661.0

MIN_PHI = np.array([50.0, 0.0075], np.float32)
DPHI = np.array([40.0, 0.0492], np.float32)
MU = np.array([70.0, 0.032], np.float32)
SIG = np.array([20.0, 0.025], np.float32)

# matmul dtype: "f32" (4 cyc/row, exact), "f32r" (1 cyc/row, ~19-bit mantissa)
MODE = os.environ.get("BASS_KERNEL_MODE", "f32r")

# packed parameters: pp (2 partitions: phiT|W1), pk (128p: W2|W3|W4),
# pb (128p fp32: b1|b2|b3|b4', with ln(NORM) folded into b4')
PP_COLS = 612
PK_COLS = 328
PB_COLS = 4

_CACHE = {}


def _spline_eval_matrix(wn_vals):
    """E (len(wn_vals), 128) fp32: natural-cubic-spline evaluation at wn_vals,
    linear in the 128 knot values (knots t_k = sqrt(2)*k in fp32)."""
    wn = (256.0 * np.fft.fftfreq(256, d=1.0)).reshape(256, 1)
    wn_iso = np.sqrt(wn**2 + wn.reshape(1, 256) ** 2)
    t32 = np.fft.fftshift(wn_iso).diagonal()[128:].astype(np.float32)  # (128,)

    n = 128
    t = t32.astype(np.float64)
    h = np.diff(t)
    A = np.diag(2.0 * (h[:-1] + h[1:])) + np.diag(h[1:-1], 1) + np.diag(h[1:-1], -1)
    D1 = np.zeros((n - 1, n))
    for i in range(n - 1):
        D1[i, i] = -1.0 / h[i]
        D1[i, i + 1] = 1.0 / h[i]
    D2 = 6.0 * (D1[1:] - D1[:-1])
    L = np.zeros((n, n))
    L[1:-1] = np.linalg.solve(A, D2)

    Sa = np.eye(n)[: n - 1]
    Sb = D1 - (h[:, None] / 6.0) * (2.0 * L[:-1] + L[1:])
    Sc = L[:-1] / 2.0
    Sd = (L[1:] - L[:-1]) / (6.0 * h[:, None])

    w32 = wn_vals.astype(np.float32)
    idx = np.clip(np.searchsorted(t32, w32, side="right") - 1, 0, n - 2)
    f = (w32 - t32[idx]).astype(np.float64)[:, None]
    E = Sa[idx] + f * (Sb[idx] + f * (Sc[idx] + f * Sd[idx]))
    return E.astype(np.float32)


def _build_constants():
    """ET_u (128, P_PAD) fp32 for the unique values, and IDX (65536,) int32
    mapping each full-grid point to its unique-value column."""
    k = np.arange(256)
    absw = np.minimum(k, 256 - k)  # |wn_i|, with |wn_0| = 0, |wn_128| = 128
    s = absw[:, None].astype(np.int64) ** 2 + absw[None, :].astype(np.int64) ** 2
    uniq_s, inv = np.unique(s.ravel(), return_inverse=True)  # (N_UNIQ,), (65536,)

    wn_vals = np.sqrt(uniq_s.astype(np.float64))
    E = _spline_eval_matrix(wn_vals)  # (N_UNIQ, 128)
    ET = np.zeros((128, P_PAD), np.float32)
    ET[:, :N_UNIQ] = E.T
    return np.ascontiguousarray(ET), inv.astype(np.int32)


def _build_program(mode):
    import concourse.bass as bass
    import concourse.bacc as bacc
    import concourse.mybir as mybir
    from concourse import tile

    f32 = mybir.dt.float32
    bf16 = mybir.dt.bfloat16
    mm_dt = {"f32r": mybir.dt.float32r, "f32": f32}[mode]
    nc = bacc.Bacc("TRN2", target_bir_lowering=False, debug=False)

    pp_d = nc.dram_tensor("pp", [2, PP_COLS], mm_dt, kind="ExternalInput")
    pk_d = nc.dram_tensor("pk", [128, PK_COLS], mm_dt, kind="ExternalInput")
    pb_d = nc.dram_tensor("pb", [128, PB_COLS], f32, kind="ExternalInput")
    et_d = nc.dram_tensor("et", [128, P_CORE], mm_dt, kind="ExternalInput")
    out_d = nc.dram_tensor("out", [128, 4 * P_CORE], bf16, kind="ExternalOutput")

    Relu = mybir.ActivationFunctionType.Relu
    Exp = mybir.ActivationFunctionType.Exp

    N_GRP = 4
    SUB = 512  # matmul free chunk (f32 moving-operand limit)
    HBW = 256  # MLP half-batch width (psum tile size)

    with tile.TileContext(nc) as tc:
        with (
            tc.tile_pool(name="const", bufs=1) as cpool,
            tc.tile_pool(name="mlp", bufs=2) as mpool,
            tc.tile_pool(name="stage", bufs=4) as spool,
            tc.tile_pool(name="psum", bufs=2, space=bass.MemorySpace.PSUM) as ppool,
            tc.tile_pool(name="mpsum", bufs=2, space=bass.MemorySpace.PSUM) as mps,
        ):
            # ---- loads: pk+stores on the SP ring; phi, biases and ET on the
            # ACT ring (phi first so layer 1 starts earliest; pk lands in
            # parallel on the other ring well before layer 2 needs it) ----
            pk_t = cpool.tile([128, PK_COLS], mm_dt, tag="pk")
            nc.sync.dma_start(pk_t[:], pk_d[:])

            # warm-up matmuls on zeros: keeps the PE busy through the load
            # window so the HAM clock gate releases (1.2 -> 2.4 GHz) before
            # the real matmuls run
            z_t = cpool.tile([128, 256], f32, tag="z")
            nc.vector.memset(z_t[:], 0.0)
            z_r = cpool.tile([128, 256], mm_dt, tag="zr")
            nc.vector.tensor_scalar(
                z_r[:], z_t[:], 0.0, None, mybir.AluOpType.add
            )
            zp = mps.tile([128, HBW], f32, tag="mps")
            for _ in range(3):
                nc.tensor.matmul(zp[:], z_t[0:128, 0:128], z_t[:])
            pp_t = cpool.tile([2, PP_COLS], mm_dt, tag="pp")
            nc.scalar.dma_start(pp_t[:], pp_d[:])
            pb_t = cpool.tile([128, PB_COLS], f32, tag="pb")
            nc.scalar.dma_start(pb_t[:], pb_d[:])
            et_t = cpool.tile([128, P_CORE], mm_dt, tag="et")
            nc.scalar.dma_start(et_t[:], et_d[:])

            pht = pp_t[0:2, 0:512]
            w1 = pp_t[0:2, 512:612]
            w2 = pk_t[0:100, 0:100]
            w3 = pk_t[0:100, 100:200]
            w4 = pk_t[0:100, 200:328]
            b1 = pb_t[0:100, 0:1]
            b2 = pb_t[0:100, 1:2]
            b3 = pb_t[0:100, 2:3]
            b4 = pb_t[0:128, 3:4]  # includes ln(NORM) fold

            # ---- MLP, two interleaved 256-wide chains (hides sem latency) ----
            HB = B // 2
            diag = mpool.tile([128, B], mm_dt, tag="diag")
            hs = {}
            for lyr, (wt, bt, act, win, wout) in enumerate(
                [
                    (w1, b1, Relu, 2, 100),
                    (w2, b2, Relu, 100, 100),
                    (w3, b3, Relu, 100, 100),
                    (w4, b4, None, 100, 128),
                ]
            ):
                for c in range(2):
                    cs = slice(c * HB, (c + 1) * HB)
                    src = pht[:, cs] if lyr == 0 else hs[c][:]
                    ps = mps.tile([128, HBW], f32, tag="mps")
                    nc.tensor.matmul(ps[0:wout, 0:HB], wt, src)
                    if lyr < 3:
                        h = mpool.tile([100, HB], mm_dt, tag=f"h{lyr}{c}")
                        nc.vector.tensor_scalar(
                            h[:], ps[0:wout, 0:HB], bt, 0.0,
                            mybir.AluOpType.add, mybir.AluOpType.max,
                        )
                        hs[c] = h
                    else:
                        nc.vector.tensor_scalar(
                            diag[:, cs], ps[0:wout, 0:HB], bt, None,
                            mybir.AluOpType.add,
                        )


            # ---- main: out[g] = exp(diag_g.T @ ET_u), one bf16 store per g.
            # Filler matmuls plug the PE idle gaps (psum WAR on exp) so the
            # HAM clock gate stays released through the group pipeline. ----
            for g in range(N_GRP):
                ps = ppool.tile([128, P_CORE], f32, tag="ps")
                for off in range(0, P_CORE, SUB):
                    w = min(SUB, P_CORE - off)
                    nc.tensor.matmul(
                        ps[:, off : off + w],
                        diag[:, g * 128 : (g + 1) * 128],
                        et_t[:, off : off + w],
                    )
                stage = spool.tile([128, P_CORE], bf16, tag="stage")
                nc.scalar.activation(stage[:], ps[:], Exp)
                ring = nc.scalar if g == N_GRP - 1 else nc.sync
                ring.dma_start(out_d[:, g * P_CORE : (g + 1) * P_CORE], stage[:])
                if g in (1, 2):
                    zf = mps.tile([128, HBW], f32, tag="mps")
                    nc.tensor.matmul(
                        zf[:, 0:128], z_r[0:128, 0:128], z_r[:, 0:128]
                    )

    nc.compile()
    return nc


def _get_cached():
    key = ("nc", MODE)
    if key not in _CACHE:
        _CACHE[key] = _build_program(MODE)
    if "consts" not in _CACHE:
        _CACHE["consts"] = _build_constants()
    return (_CACHE[key],) + _CACHE["consts"]


def _make_in_maps(phi, W1, b1, W2, b2, W3, b3, W4, b4, ET):
    # fold the input normalization into the first layer, ln(NORM) into b4
    scale = (DPHI / SIG).astype(np.float32)
    shift = ((MIN_PHI - MU) / SIG).astype(np.float32)
    W1f = (np.asarray(W1, np.float32) * scale[:, None]).astype(np.float32)
    b1f = (np.asarray(b1, np.float32) + shift @ np.asarray(W1, np.float32)).astype(
        np.float32
    )

    pp = np.zeros((2, PP_COLS), np.float32)
    pp[:, 0:512] = np.asarray(phi, np.float32).T
    pp[:, 512:612] = W1f
    pk = np.zeros((128, PK_COLS), np.float32)
    pk[0:100, 0:100] = np.asarray(W2, np.float32)
    pk[0:100, 100:200] = np.asarray(W3, np.float32)
    pk[0:100, 200:328] = np.asarray(W4, np.float32)
    pb = np.zeros((128, PB_COLS), np.float32)
    pb[0:100, 0] = b1f
    pb[0:100, 1] = np.asarray(b2, np.float32)
    pb[0:100, 2] = np.asarray(b3, np.float32)
    pb[0:128, 3] = np.asarray(b4, np.float32) + np.float32(
        np.log(np.float64(NORM))
    )

    common = {"pp": pp, "pk": pk, "pb": pb}
    in_maps = []
    for c in range(N_CORES):
        m = dict(common)
        m["et"] = np.ascontiguousarray(ET[:, c * P_CORE : (c + 1) * P_CORE])
        in_maps.append(m)
    return in_maps


def kernel(phi, W1, b1, W2, b2, W3, b3, W4, b4):
    from concourse.bass_utils import run_bass_kernel_spmd

    nc, ET, IDX = _get_cached()
    in_maps = _make_in_maps(phi, W1, b1, W2, b2, W3, b3, W4, b4, ET)
    res = run_bass_kernel_spmd(nc, in_maps, core_ids=list(range(N_CORES)))
    uniq = np.empty((B, P_PAD), np.float32)
    for c, r in enumerate(res.results):
        o = np.asarray(r["out"]).astype(np.float32)  # (128, 4*P_CORE) bf16
        for g in range(4):
            uniq[g * 128 : (g + 1) * 128, c * P_CORE : (c + 1) * P_CORE] = o[
                :, g * P_CORE : (g + 1) * P_CORE
            ]
    full = np.take(uniq, IDX, axis=1)  # (512, 65536) constant-gather replication
    return np.ascontiguousarray(full.reshape(B, 256, 256))
